# revision 30
# baseline (speedup 1.0000x reference)
"""Distributed GQA attention prefill for TRN2 (8 NeuronCores).

Problem: T=2048, D=4096, N=32 query heads, K=8 kv heads, H=128.
    q = x @ w_q; k = x @ w_k; v = x @ w_v   (fused in the reference)
    rope(q), rope(k); causal GQA attention; out = o @ w_o

Sharding (tensor-parallel over heads): core c owns query heads
4c..4c+3 and kv head c (GQA groups align: query heads 4c..4c+3 attend
kv head c). w_q/w_o sharded on N, w_k/w_v on K, x replicated. Each
core computes its partial o_proj output [T, D]; a bf16 ReduceScatter
(pipelined per 512-row t-panel) sums partials; the host concatenates
the per-core shards.

On-core dataflow (bf16 matmuls, fp32 PSUM accumulation):
    x --cast-DMA--> bf16 --DMA-xbar-transpose--> xT[d, t] (SBUF)
    qT/kT = w.T @ xT per head (h on partitions), RoPE applied in the
    transposed layout with host-precomputed cos/sin tables (the half
    swap is an SBUF->SBUF DMA).
    S^T[s, t] = kT_sblock.T @ qT_panel  (causal block skipping)
    P^T = exp(S^T / sqrt(H)) on ScalarE (scores are O(1); no max pass)
    PV: out[t, 0:129] = P^T_block.T @ [v | ones]  (row sums for free)
    normalize with DVE reciprocal + tensor_scalar, transpose o via
    identity matmul, o_proj accumulating over heads, per-panel RS.
"""

import numpy as np
import ml_dtypes

T, D, NH, KH, H = 2048, 4096, 32, 8, 128
THETA = 10000.0
G = NH // KH          # 4 query heads per core
N_CORES = 8
TP_SIZE = 512         # t-panel
NTP = T // TP_SIZE    # 4 t-panels
NTB = T // 128        # 16 t/s blocks
NDB = D // 128        # 32 d blocks
SCALE = 1.0 / float(np.sqrt(H))
VEXT_STRIDE = 132     # v_ext row stride (129 used, padded)
# Output ReduceScatter chunks: (global_start_row, nrows). Large chunks for
# stream throughput; small final chunks to shrink the serial tail.
RS_CHUNKS = [(0, 256), (256, 256), (512, 256), (768, 256), (1024, 256),
             (1280, 256), (1536, 256), (1792, 128), (1920, 128)]

_NC_CACHE = {}


def _build_nc():
    import concourse.mybir as mybir
    import concourse.tile as tile
    from concourse import bacc
    from concourse.masks import make_identity

    BF16 = mybir.dt.bfloat16
    F32 = mybir.dt.float32
    EXP = mybir.ActivationFunctionType.Exp

    nc = bacc.Bacc("TRN2", target_bir_lowering=False, debug=False,
                   num_devices=N_CORES)

    xt_ext = nc.dram_tensor("xt", [128, NTP, NDB, TP_SIZE], BF16,
                            kind="ExternalInput")
    wq_ext = nc.dram_tensor("wq", [128, NDB, G * H], BF16,
                            kind="ExternalInput")
    wk_ext = nc.dram_tensor("wk", [128, NDB, H], BF16, kind="ExternalInput")
    wv_ext = nc.dram_tensor("wv", [128, NDB, H], BF16, kind="ExternalInput")
    wo_ext = nc.dram_tensor("wo", [128, G, D], BF16, kind="ExternalInput")
    cos_ext = nc.dram_tensor("cos_t", [H, T], BF16, kind="ExternalInput")
    sin_ext = nc.dram_tensor("sin_t", [H, T], BF16, kind="ExternalInput")
    mask_ext = nc.dram_tensor("maskp", [128, 128], BF16, kind="ExternalInput")
    out_ext = nc.dram_tensor("out", [T // N_CORES, D], BF16,
                             kind="ExternalOutput")

    with tile.TileContext(nc) as tc:
        with (
            tc.tile_pool(name="consts", bufs=1) as consts,
            tc.tile_pool(name="persist", bufs=1) as persist,
            tc.tile_pool(name="dram", bufs=1, space="DRAM") as dram,
            tc.tile_pool(name="xw", bufs=1) as xw,
            tc.tile_pool(name="xtpp", bufs=1) as xtpp,
            tc.tile_pool(name="rawp", bufs=1) as rawp,
            tc.tile_pool(name="ropep", bufs=1) as ropep,
            tc.tile_pool(name="wop", bufs=1) as wop,
            tc.tile_pool(name="ptp", bufs=17) as ptp,
            tc.tile_pool(name="otp", bufs=5) as otp,
            tc.tile_pool(name="osbp", bufs=4) as osbp,
            tc.tile_pool(name="scp", bufs=6) as scp,
            tc.tile_pool(name="widps", bufs=2, space="PSUM") as widps,
            tc.tile_pool(name="smallps", bufs=2, space="PSUM") as smallps,
            tc.tile_pool(name="oprojps", bufs=4, space="PSUM") as oprojps,
        ):
            cos_sb = consts.tile([H, T], BF16)
            sin_sb = consts.tile([H, T], BF16)
            mask_sb = consts.tile([128, 128], BF16)
            ident = consts.tile([128, 128], BF16)
            make_identity(nc, ident[:])

            qT = [persist.tile([128, T], BF16, tag=f"qT{g}", name=f"qT{g}")
                  for g in range(G)]
            kT = persist.tile([128, T], BF16)
            v_ext = persist.tile([128, NTB, VEXT_STRIDE], BF16)
            vraw = rawp.tile([128, T], BF16, tag="raw")

            rs_in = [dram.tile([n, D], BF16, tag=f"rsi{ch}", name=f"rsi{ch}")
                     for ch, (s, n) in enumerate(RS_CHUNKS)]
            rs_out = [dram.tile([n // N_CORES, D], BF16, tag=f"rso{ch}",
                                name=f"rso{ch}")
                      for ch, (s, n) in enumerate(RS_CHUNKS)]

            wq_sb = xw.tile([128, NDB, G * H], BF16)
            wk_sb = xw.tile([128, NDB, H], BF16)
            wv_sb = xw.tile([128, NDB, H], BF16)
            wo_sb = wop.tile([128, G, D], BF16)
            nc.scalar.dma_start(out=wk_sb[:], in_=wk_ext[:])
            nc.scalar.dma_start(out=wv_sb[:], in_=wv_ext[:])
            for cch in range(4):
                nc.scalar.dma_start(
                    out=wq_sb[:, 8 * cch:8 * (cch + 1), :],
                    in_=wq_ext[:, 8 * cch:8 * (cch + 1), :])
            nc.gpsimd.dma_start(out=cos_sb[:], in_=cos_ext[:])
            nc.gpsimd.dma_start(out=sin_sb[:], in_=sin_ext[:])
            nc.gpsimd.dma_start(out=mask_sb[:], in_=mask_ext[:])
            nc.gpsimd.dma_start(out=wo_sb[:], in_=wo_ext[:])
            nc.vector.memset(v_ext[:, :, 128:129], 1.0)

            def rope(raw, dst, tsl):
                """dst = raw*cos + halfswap(raw)*sin over one panel."""
                sw = ropep.tile([128, TP_SIZE], BF16, tag="ropesw")
                t1 = ropep.tile([128, TP_SIZE], BF16, tag="ropet1")
                nc.gpsimd.dma_start(out=sw[0:64, :], in_=raw[64:128, :])
                nc.gpsimd.dma_start(out=sw[64:128, :], in_=raw[0:64, :])
                nc.vector.tensor_tensor(
                    out=t1[:], in0=raw[:], in1=cos_sb[:, tsl],
                    op=mybir.AluOpType.mult)
                nc.vector.tensor_tensor(
                    out=sw[:], in0=sw[:], in1=sin_sb[:, tsl],
                    op=mybir.AluOpType.mult)
                nc.vector.tensor_tensor(
                    out=dst[:], in0=t1[:], in1=sw[:],
                    op=mybir.AluOpType.add)

            def oproj(tp, j, oT_t):
                """o_proj for t-block j of panel tp + its ReduceScatter."""
                tb = 4 * tp + j
                ch = next(i for i, (s, n) in enumerate(RS_CHUNKS)
                          if s <= tb * 128 < s + n)
                row = tb * 128 - RS_CHUNKS[ch][0]
                for dh in range(2):
                    osb = osbp.tile([128, D // 2], BF16, tag="osb")
                    ods = [oprojps.tile([128, 512], F32, tag="od",
                                        name=f"od{tp}_{j}_{dh}_{ii}")
                           for ii in range(4)]
                    for g in range(G):
                        for i, dp in enumerate(range(dh * 4, dh * 4 + 4)):
                            nc.tensor.matmul(
                                ods[i][:],
                                oT_t[g][:, j * 128:(j + 1) * 128],
                                wo_sb[:, g, dp * 512:(dp + 1) * 512],
                                start=(g == 0), stop=(g == G - 1),
                                skip_group_check=True)
                    for i, dp in enumerate(range(dh * 4, dh * 4 + 4)):
                        if tp == NTP - 1 and dp % 2 == 1:
                            nc.scalar.copy(
                                osb[:, (dp - dh * 4) * 512:
                                    (dp - dh * 4 + 1) * 512], ods[i][:])
                        else:
                            nc.vector.tensor_copy(
                                osb[:, (dp - dh * 4) * 512:
                                    (dp - dh * 4 + 1) * 512], ods[i][:])
                    nc.sync.dma_start(
                        out=rs_in[ch][row:row + 128,
                                      dh * 2048:(dh + 1) * 2048],
                        in_=osb[:])
                if row + 128 == RS_CHUNKS[ch][1]:
                    nc.gpsimd.collective_compute(
                        "ReduceScatter",
                        mybir.AluOpType.add,
                        replica_groups=[list(range(N_CORES))],
                        ins=[rs_in[ch].opt()],
                        outs=[rs_out[ch].opt()],
                    )
                    s, n = RS_CHUNKS[ch]
                    nc.gpsimd.dma_start(
                        out=out_ext[s // N_CORES:
                                    s // N_CORES + n // N_CORES, :],
                        in_=rs_out[ch][:])

            def attention(tp):
                n_sb = 4 * tp + 4
                oT_t = [otp.tile([128, TP_SIZE], BF16, tag="oT",
                                 name=f"oT{tp}_{gg}")
                        for gg in range(G)]
                for g in range(G):
                    pts = []
                    for sb in range(n_sb):
                        ps_s = widps.tile([128, TP_SIZE], F32, tag="wide")
                        nc.tensor.matmul(
                            ps_s[:], kT[:, sb * 128:(sb + 1) * 128],
                            qT[g][:, tp * TP_SIZE:(tp + 1) * TP_SIZE],
                            start=True, stop=True)
                        pt = ptp.tile([128, TP_SIZE], BF16, tag="pt")
                        nc.scalar.activation(pt[:], ps_s[:], EXP, scale=SCALE)
                        jj = sb - 4 * tp
                        if 0 <= jj:
                            nc.vector.tensor_tensor(
                                out=pt[:, jj * 128:(jj + 1) * 128],
                                in0=pt[:, jj * 128:(jj + 1) * 128],
                                in1=mask_sb[:],
                                op=mybir.AluOpType.mult)
                        pts.append(pt)
                    for j in range(4):
                        tb = 4 * tp + j
                        ps_pv = smallps.tile([128, 129], F32, tag="sm")
                        for sb in range(tb + 1):
                            nc.tensor.matmul(
                                ps_pv[:],
                                pts[sb][:, j * 128:(j + 1) * 128],
                                v_ext[:, sb, 0:129],
                                start=(sb == 0), stop=(sb == tb),
                                skip_group_check=True)
                        rc = scp.tile([128, 1], F32, tag="rc")
                        nc.vector.reciprocal(rc[:], ps_pv[:, 128:129])
                        ob = scp.tile([128, 128], BF16, tag="ob")
                        nc.vector.tensor_scalar_mul(
                            ob[:], ps_pv[:, 0:128], rc[:])
                        ps_tr = smallps.tile([128, 128], BF16, tag="sm")
                        nc.tensor.transpose(ps_tr[:], ob[:], ident[:])
                        nc.vector.tensor_copy(
                            oT_t[g][:, j * 128:(j + 1) * 128], ps_tr[:])
                        if g == G - 1:
                            oproj(tp, j, oT_t)

            for tp in range(NTP):
                tsl = slice(tp * TP_SIZE, (tp + 1) * TP_SIZE)
                xTp = xtpp.tile([128, NDB, TP_SIZE], BF16, tag="xTp")
                if tp == 0:
                    for dc in range(4):
                        nc.sync.dma_start(
                            out=xTp[:, 8 * dc:8 * (dc + 1), :],
                            in_=xt_ext[:, 0, 8 * dc:8 * (dc + 1), :])
                else:
                    nc.sync.dma_start(out=xTp[:], in_=xt_ext[:, tp])
                ps = widps.tile([128, TP_SIZE], F32, tag="wide")
                for db in range(NDB):
                    nc.tensor.matmul(
                        ps[:], wk_sb[:, db, :], xTp[:, db, :],
                        start=(db == 0), stop=(db == NDB - 1))
                nc.scalar.copy(kT[:, tsl], ps[:])
                rope(kT[:, tsl], kT[:, tsl], tsl)
                ps = widps.tile([128, TP_SIZE], F32, tag="wide")
                for db in range(NDB):
                    nc.tensor.matmul(
                        ps[:], wv_sb[:, db, :], xTp[:, db, :],
                        start=(db == 0), stop=(db == NDB - 1))
                nc.scalar.copy(vraw[:, tsl], ps[:])
                for tl in range(4):
                    sb = 4 * tp + tl
                    pst = smallps.tile([128, 128], BF16, tag="sm")
                    nc.tensor.transpose(
                        pst[:], vraw[:, sb * 128:(sb + 1) * 128], ident[:])
                    nc.vector.tensor_copy(v_ext[:, sb, 0:128], pst[:])
                for g in range(G):
                    ps = widps.tile([128, TP_SIZE], F32, tag="wide")
                    for db in range(NDB):
                        nc.tensor.matmul(
                            ps[:],
                            wq_sb[:, db, g * H:(g + 1) * H],
                            xTp[:, db, :],
                            start=(db == 0), stop=(db == NDB - 1))
                    nc.scalar.copy(qT[g][:, tsl], ps[:])
                    rope(qT[g][:, tsl], qT[g][:, tsl], tsl)
                if tp > 0:
                    attention(tp - 1)
            attention(NTP - 1)

    nc.compile()
    return nc


def get_nc():
    if "nc" not in _NC_CACHE:
        _NC_CACHE["nc"] = _build_nc()
    return _NC_CACHE["nc"]


def make_in_maps(x, positions, w_q, w_k, w_v, w_o):
    """Host-side sharding + RoPE table / mask precompute."""
    x = np.ascontiguousarray(np.asarray(x, np.float32))
    positions = np.asarray(positions)

    half = H // 2
    inv_freq = 1.0 / (THETA ** (np.arange(half, dtype=np.float32) / half))
    ang = positions.astype(np.float32)[:, None] * inv_freq[None, :]  # [T, 64]
    cos = np.cos(ang)   # [T, 64]
    sin = np.sin(ang)
    cos_t = np.empty((H, T), np.float32)
    sin_t = np.empty((H, T), np.float32)
    cos_t[0:half] = cos.T
    cos_t[half:] = cos.T
    sin_t[0:half] = -sin.T
    sin_t[half:] = sin.T
    cos_t = cos_t.astype(ml_dtypes.bfloat16)
    sin_t = sin_t.astype(ml_dtypes.bfloat16)

    # mask[s, t] = 1 if s <= t (lower-left of P^T allowed region)
    idx = np.arange(128)
    maskp = (idx[:, None] <= idx[None, :]).astype(ml_dtypes.bfloat16)

    xt = x.astype(ml_dtypes.bfloat16).T  # [D, T]
    xt4 = np.ascontiguousarray(
        xt.reshape(NDB, 128, NTP, TP_SIZE).transpose(1, 2, 0, 3))
    w_q = np.asarray(w_q, np.float32).reshape(D, NH, H).astype(
        ml_dtypes.bfloat16)
    w_k = np.asarray(w_k, np.float32).reshape(D, KH, H).astype(
        ml_dtypes.bfloat16)
    w_v = np.asarray(w_v, np.float32).reshape(D, KH, H).astype(
        ml_dtypes.bfloat16)
    w_o = np.asarray(w_o, np.float32).reshape(NH, H, D).astype(
        ml_dtypes.bfloat16)

    def blk(w):
        """[D, n] -> [128, NDB, n] with row d = a*128 + p."""
        return np.ascontiguousarray(
            w.reshape(NDB, 128, -1).transpose(1, 0, 2))

    in_maps = []
    for c in range(N_CORES):
        in_maps.append({
            "xt": xt4,
            "wq": blk(w_q[:, G * c:G * (c + 1), :].reshape(D, G * H)),
            "wk": blk(w_k[:, c, :]),
            "wv": blk(w_v[:, c, :]),
            "wo": np.ascontiguousarray(
                w_o[G * c:G * (c + 1)].reshape(G, 128, D)
                .transpose(1, 0, 2)),
            "cos_t": cos_t,
            "sin_t": sin_t,
            "maskp": maskp,
        })
    return in_maps


def assemble_output(results):
    """results: list of 8 per-core dicts with 'out' [T//8, D] bf16."""
    out = np.empty((T, D), np.float32)
    for c in range(N_CORES):
        o = np.asarray(results[c]["out"], np.float32)
        for s, n in RS_CHUNKS:
            k = n // N_CORES
            out[s + c * k:s + (c + 1) * k] = o[s // N_CORES:s // N_CORES + k]
    return out


def kernel(x, positions, w_q, w_k, w_v, w_o):
    from concourse.bass_utils import run_bass_kernel_spmd

    nc = get_nc()
    in_maps = make_in_maps(x, positions, w_q, w_k, w_v, w_o)
    res = run_bass_kernel_spmd(nc, in_maps, core_ids=list(range(N_CORES)))
    return assemble_output(res.results)


# revision 32
# speedup vs baseline: 1.0286x; 1.0286x over previous
"""Distributed GQA attention prefill for TRN2 (8 NeuronCores).

Problem: T=2048, D=4096, N=32 query heads, K=8 kv heads, H=128.
    q = x @ w_q; k = x @ w_k; v = x @ w_v   (fused in the reference)
    rope(q), rope(k); causal GQA attention; out = o @ w_o

Sharding (tensor-parallel over heads): core c owns query heads
4c..4c+3 and kv head c (GQA groups align: query heads 4c..4c+3 attend
kv head c). w_q/w_o sharded on N, w_k/w_v on K, x replicated. Each
core computes its partial o_proj output [T, D]; a bf16 ReduceScatter
(pipelined per 512-row t-panel) sums partials; the host concatenates
the per-core shards.

On-core dataflow (bf16 matmuls, fp32 PSUM accumulation):
    x --cast-DMA--> bf16 --DMA-xbar-transpose--> xT[d, t] (SBUF)
    qT/kT = w.T @ xT per head (h on partitions), RoPE applied in the
    transposed layout with host-precomputed cos/sin tables (the half
    swap is an SBUF->SBUF DMA).
    S^T[s, t] = kT_sblock.T @ qT_panel  (causal block skipping)
    P^T = exp(S^T / sqrt(H)) on ScalarE (scores are O(1); no max pass)
    PV: out[t, 0:129] = P^T_block.T @ [v | ones]  (row sums for free)
    normalize with DVE reciprocal + tensor_scalar, transpose o via
    identity matmul, o_proj accumulating over heads, per-panel RS.
"""

import numpy as np
import ml_dtypes

T, D, NH, KH, H = 2048, 4096, 32, 8, 128
THETA = 10000.0
G = NH // KH          # 4 query heads per core
N_CORES = 8
TP_SIZE = 512         # t-panel
NTP = T // TP_SIZE    # 4 t-panels
NTB = T // 128        # 16 t/s blocks
NDB = D // 128        # 32 d blocks
SCALE = 1.0 / float(np.sqrt(H))
VEXT_STRIDE = 132     # v_ext row stride (129 used, padded)
# Output ReduceScatter chunks: (global_start_row, nrows). Large chunks for
# stream throughput; small final chunks to shrink the serial tail.
RS_CHUNKS = [(0, 256), (256, 256), (512, 256), (768, 256), (1024, 256),
             (1280, 256), (1536, 128), (1664, 128), (1792, 128), (1920, 128)]

_NC_CACHE = {}


def _build_nc():
    import concourse.mybir as mybir
    import concourse.tile as tile
    from concourse import bacc
    from concourse.masks import make_identity

    BF16 = mybir.dt.bfloat16
    F32 = mybir.dt.float32
    EXP = mybir.ActivationFunctionType.Exp

    nc = bacc.Bacc("TRN2", target_bir_lowering=False, debug=False,
                   num_devices=N_CORES)

    xt_ext = nc.dram_tensor("xt", [128, NTP, NDB, TP_SIZE], BF16,
                            kind="ExternalInput")
    wq_ext = nc.dram_tensor("wq", [128, NDB, G * H], BF16,
                            kind="ExternalInput")
    wk_ext = nc.dram_tensor("wk", [128, NDB, H], BF16, kind="ExternalInput")
    wv_ext = nc.dram_tensor("wv", [128, NDB, H], BF16, kind="ExternalInput")
    wo_ext = nc.dram_tensor("wo", [128, G, D], BF16, kind="ExternalInput")
    cos_ext = nc.dram_tensor("cos_t", [H, T], BF16, kind="ExternalInput")
    sin_ext = nc.dram_tensor("sin_t", [H, T], BF16, kind="ExternalInput")
    mask_ext = nc.dram_tensor("maskp", [128, 128], BF16, kind="ExternalInput")
    out_ext = nc.dram_tensor("out", [T // N_CORES, D], BF16,
                             kind="ExternalOutput")

    with tile.TileContext(nc) as tc:
        with (
            tc.tile_pool(name="consts", bufs=1) as consts,
            tc.tile_pool(name="persist", bufs=1) as persist,
            tc.tile_pool(name="dram", bufs=1, space="DRAM") as dram,
        ):
            cos_sb = consts.tile([H, T], BF16)
            sin_sb = consts.tile([H, T], BF16)
            mask_sb = consts.tile([128, 128], BF16)
            ident = consts.tile([128, 128], BF16)
            make_identity(nc, ident[:])

            qT = [persist.tile([128, T], BF16, tag=f"qT{g}", name=f"qT{g}")
                  for g in range(G)]
            kT = persist.tile([128, T], BF16)
            v_ext = persist.tile([128, NTB, VEXT_STRIDE], BF16)

            rs_in = [dram.tile([n, D], BF16, tag=f"rsi{ch}", name=f"rsi{ch}")
                     for ch, (s, n) in enumerate(RS_CHUNKS)]
            rs_out = [dram.tile([n // N_CORES, D], BF16, tag=f"rso{ch}",
                                name=f"rso{ch}")
                      for ch, (s, n) in enumerate(RS_CHUNKS)]

            # ============ phase A: x transpose + QKV + RoPE ============
            with (
                tc.tile_pool(name="xw", bufs=1) as xw,
                tc.tile_pool(name="xtpp", bufs=2) as xtpp,
                tc.tile_pool(name="rawp", bufs=1) as rawp,
                tc.tile_pool(name="ropep", bufs=1) as ropep,
                tc.tile_pool(name="qkvps", bufs=4, space="PSUM") as qkv_ps,
                tc.tile_pool(name="vtrps", bufs=2, space="PSUM") as vtr_ps,
            ):
                wq_sb = xw.tile([128, NDB, G * H], BF16)
                wk_sb = xw.tile([128, NDB, H], BF16)
                wv_sb = xw.tile([128, NDB, H], BF16)
                vraw = rawp.tile([128, T], BF16, tag="raw")

                nc.scalar.dma_start(out=wk_sb[:], in_=wk_ext[:])
                nc.scalar.dma_start(out=wv_sb[:], in_=wv_ext[:])
                for cch in range(4):
                    nc.scalar.dma_start(
                        out=wq_sb[:, 8 * cch:8 * (cch + 1), :],
                        in_=wq_ext[:, 8 * cch:8 * (cch + 1), :])
                nc.gpsimd.dma_start(out=cos_sb[:], in_=cos_ext[:])
                nc.gpsimd.dma_start(out=sin_sb[:], in_=sin_ext[:])
                nc.gpsimd.dma_start(out=mask_sb[:], in_=mask_ext[:])

                def rope(raw, dst):
                    """dst = raw*cos + halfswap(raw)*sin (full [128, T])."""
                    sw = ropep.tile([128, T], BF16, tag="ropesw")
                    t1 = ropep.tile([128, T], BF16, tag="ropet1")
                    nc.gpsimd.dma_start(out=sw[0:64, :], in_=raw[64:128, :])
                    nc.gpsimd.dma_start(out=sw[64:128, :], in_=raw[0:64, :])
                    nc.vector.tensor_tensor(
                        out=t1[:], in0=raw[:], in1=cos_sb[:],
                        op=mybir.AluOpType.mult)
                    nc.vector.tensor_tensor(
                        out=sw[:], in0=sw[:], in1=sin_sb[:],
                        op=mybir.AluOpType.mult)
                    nc.vector.tensor_tensor(
                        out=dst[:], in0=t1[:], in1=sw[:],
                        op=mybir.AluOpType.add)

                for tp in range(NTP):
                    xTp = xtpp.tile([128, NDB, TP_SIZE], BF16, tag="xTp")
                    if tp == 0:
                        for dc in range(4):
                            nc.sync.dma_start(
                                out=xTp[:, 8 * dc:8 * (dc + 1), :],
                                in_=xt_ext[:, 0, 8 * dc:8 * (dc + 1), :])
                    else:
                        nc.sync.dma_start(out=xTp[:], in_=xt_ext[:, tp])
                    tsl = slice(tp * TP_SIZE, (tp + 1) * TP_SIZE)
                    ps = qkv_ps.tile([128, TP_SIZE], F32, tag="qkv")
                    for db in range(NDB):
                        nc.tensor.matmul(
                            ps[:], wk_sb[:, db, :], xTp[:, db, :],
                            start=(db == 0), stop=(db == NDB - 1))
                    nc.scalar.copy(kT[:, tsl], ps[:])
                    ps = qkv_ps.tile([128, TP_SIZE], F32, tag="qkv")
                    for db in range(NDB):
                        nc.tensor.matmul(
                            ps[:], wv_sb[:, db, :], xTp[:, db, :],
                            start=(db == 0), stop=(db == NDB - 1))
                    nc.scalar.copy(vraw[:, tsl], ps[:])
                    for g in range(G):
                        ps = qkv_ps.tile([128, TP_SIZE], F32, tag="qkv")
                        for db in range(NDB):
                            nc.tensor.matmul(
                                ps[:],
                                wq_sb[:, db, g * H:(g + 1) * H],
                                xTp[:, db, :],
                                start=(db == 0), stop=(db == NDB - 1))
                        nc.scalar.copy(qT[g][:, tsl], ps[:])

                rope(kT, kT)
                for g in range(G):
                    rope(qT[g], qT[g])
                nc.vector.memset(v_ext[:, :, 128:129], 1.0)
                for sb in range(NTB):
                    pst = vtr_ps.tile([128, 128], BF16, tag="vtr")
                    nc.tensor.transpose(
                        pst[:], vraw[:, sb * 128:(sb + 1) * 128], ident[:])
                    nc.vector.tensor_copy(v_ext[:, sb, 0:128], pst[:])

            # ============ phase B: attention + o_proj + RS ============
            with (
                tc.tile_pool(name="wop", bufs=1) as wop,
                tc.tile_pool(name="ptp", bufs=20) as ptp,
                tc.tile_pool(name="otp", bufs=8) as otp,
                tc.tile_pool(name="osbp", bufs=4) as osbp,
                tc.tile_pool(name="scp", bufs=6) as scp,
                tc.tile_pool(name="sps", bufs=2, space="PSUM") as sps,
                tc.tile_pool(name="smallps", bufs=2, space="PSUM") as smallps,
                tc.tile_pool(name="oprojps", bufs=4, space="PSUM") as oprojps,
            ):
                wo_sb = wop.tile([128, G, D], BF16)
                nc.gpsimd.dma_start(out=wo_sb[:], in_=wo_ext[:])

                def oproj(tp, j, oT_t):
                    """o_proj for t-block j of panel tp + its ReduceScatter."""
                    tb = 4 * tp + j
                    ch = next(i for i, (s, n) in enumerate(RS_CHUNKS)
                              if s <= tb * 128 < s + n)
                    row = tb * 128 - RS_CHUNKS[ch][0]
                    for dh in range(2):
                        osb = osbp.tile([128, D // 2], BF16, tag="osb")
                        ods = [oprojps.tile([128, 512], F32, tag="od",
                                            name=f"od{tp}_{j}_{dh}_{ii}")
                               for ii in range(4)]
                        for g in range(G):
                            for i, dp in enumerate(range(dh * 4, dh * 4 + 4)):
                                nc.tensor.matmul(
                                    ods[i][:],
                                    oT_t[g][:, j * 128:(j + 1) * 128],
                                    wo_sb[:, g, dp * 512:(dp + 1) * 512],
                                    start=(g == 0), stop=(g == G - 1),
                                    skip_group_check=True)
                        for i, dp in enumerate(range(dh * 4, dh * 4 + 4)):
                            if tp == NTP - 1 and dp % 2 == 1:
                                nc.scalar.copy(
                                    osb[:, (dp - dh * 4) * 512:
                                        (dp - dh * 4 + 1) * 512], ods[i][:])
                            else:
                                nc.vector.tensor_copy(
                                    osb[:, (dp - dh * 4) * 512:
                                        (dp - dh * 4 + 1) * 512], ods[i][:])
                        nc.sync.dma_start(
                            out=rs_in[ch][row:row + 128,
                                          dh * 2048:(dh + 1) * 2048],
                            in_=osb[:])
                    if row + 128 == RS_CHUNKS[ch][1]:
                        nc.gpsimd.collective_compute(
                            "ReduceScatter",
                            mybir.AluOpType.add,
                            replica_groups=[list(range(N_CORES))],
                            ins=[rs_in[ch].opt()],
                            outs=[rs_out[ch].opt()],
                        )
                        s, n = RS_CHUNKS[ch]
                        nc.gpsimd.dma_start(
                            out=out_ext[s // N_CORES:
                                        s // N_CORES + n // N_CORES, :],
                            in_=rs_out[ch][:])

                for tp in range(NTP):
                    n_sb = 4 * tp + 4
                    oT_t = [otp.tile([128, TP_SIZE], BF16, tag="oT",
                                     name=f"oT{tp}_{gg}")
                            for gg in range(G)]
                    for g in range(G):
                        pts = []
                        for sb in range(n_sb):
                            ps_s = sps.tile([128, TP_SIZE], F32, tag="s")
                            nc.tensor.matmul(
                                ps_s[:], kT[:, sb * 128:(sb + 1) * 128],
                                qT[g][:, tp * TP_SIZE:(tp + 1) * TP_SIZE],
                                start=True, stop=True)
                            pt = ptp.tile([128, TP_SIZE], BF16, tag="pt")
                            nc.scalar.activation(pt[:], ps_s[:], EXP,
                                                 scale=SCALE)
                            jj = sb - 4 * tp
                            if 0 <= jj:
                                nc.vector.tensor_tensor(
                                    out=pt[:, jj * 128:(jj + 1) * 128],
                                    in0=pt[:, jj * 128:(jj + 1) * 128],
                                    in1=mask_sb[:],
                                    op=mybir.AluOpType.mult)
                            pts.append(pt)
                        for j in range(4):
                            tb = 4 * tp + j
                            ps_pv = smallps.tile([128, 129], F32, tag="sm")
                            for sb in range(tb + 1):
                                nc.tensor.matmul(
                                    ps_pv[:],
                                    pts[sb][:, j * 128:(j + 1) * 128],
                                    v_ext[:, sb, 0:129],
                                    start=(sb == 0), stop=(sb == tb),
                                    skip_group_check=True)
                            rc = scp.tile([128, 1], F32, tag="rc")
                            nc.vector.reciprocal(rc[:], ps_pv[:, 128:129])
                            ob = scp.tile([128, 128], BF16, tag="ob")
                            nc.vector.tensor_scalar_mul(
                                ob[:], ps_pv[:, 0:128], rc[:])
                            ps_tr = smallps.tile([128, 128], BF16, tag="sm")
                            nc.tensor.transpose(ps_tr[:], ob[:], ident[:])
                            nc.vector.tensor_copy(
                                oT_t[g][:, j * 128:(j + 1) * 128], ps_tr[:])
                            if g == G - 1:
                                oproj(tp, j, oT_t)

    nc.compile()
    return nc


def get_nc():
    if "nc" not in _NC_CACHE:
        _NC_CACHE["nc"] = _build_nc()
    return _NC_CACHE["nc"]


def make_in_maps(x, positions, w_q, w_k, w_v, w_o):
    """Host-side sharding + RoPE table / mask precompute."""
    x = np.ascontiguousarray(np.asarray(x, np.float32))
    positions = np.asarray(positions)

    half = H // 2
    inv_freq = 1.0 / (THETA ** (np.arange(half, dtype=np.float32) / half))
    ang = positions.astype(np.float32)[:, None] * inv_freq[None, :]  # [T, 64]
    cos = np.cos(ang)   # [T, 64]
    sin = np.sin(ang)
    cos_t = np.empty((H, T), np.float32)
    sin_t = np.empty((H, T), np.float32)
    cos_t[0:half] = cos.T
    cos_t[half:] = cos.T
    sin_t[0:half] = -sin.T
    sin_t[half:] = sin.T
    cos_t = cos_t.astype(ml_dtypes.bfloat16)
    sin_t = sin_t.astype(ml_dtypes.bfloat16)

    # mask[s, t] = 1 if s <= t (lower-left of P^T allowed region)
    idx = np.arange(128)
    maskp = (idx[:, None] <= idx[None, :]).astype(ml_dtypes.bfloat16)

    xt = x.astype(ml_dtypes.bfloat16).T  # [D, T]
    xt4 = np.ascontiguousarray(
        xt.reshape(NDB, 128, NTP, TP_SIZE).transpose(1, 2, 0, 3))
    w_q = np.asarray(w_q, np.float32).reshape(D, NH, H).astype(
        ml_dtypes.bfloat16)
    w_k = np.asarray(w_k, np.float32).reshape(D, KH, H).astype(
        ml_dtypes.bfloat16)
    w_v = np.asarray(w_v, np.float32).reshape(D, KH, H).astype(
        ml_dtypes.bfloat16)
    w_o = np.asarray(w_o, np.float32).reshape(NH, H, D).astype(
        ml_dtypes.bfloat16)

    def blk(w):
        """[D, n] -> [128, NDB, n] with row d = a*128 + p."""
        return np.ascontiguousarray(
            w.reshape(NDB, 128, -1).transpose(1, 0, 2))

    in_maps = []
    for c in range(N_CORES):
        in_maps.append({
            "xt": xt4,
            "wq": blk(w_q[:, G * c:G * (c + 1), :].reshape(D, G * H)),
            "wk": blk(w_k[:, c, :]),
            "wv": blk(w_v[:, c, :]),
            "wo": np.ascontiguousarray(
                w_o[G * c:G * (c + 1)].reshape(G, 128, D)
                .transpose(1, 0, 2)),
            "cos_t": cos_t,
            "sin_t": sin_t,
            "maskp": maskp,
        })
    return in_maps


def assemble_output(results):
    """results: list of 8 per-core dicts with 'out' [T//8, D] bf16."""
    out = np.empty((T, D), np.float32)
    for c in range(N_CORES):
        o = np.asarray(results[c]["out"], np.float32)
        for s, n in RS_CHUNKS:
            k = n // N_CORES
            out[s + c * k:s + (c + 1) * k] = o[s // N_CORES:s // N_CORES + k]
    return out


def kernel(x, positions, w_q, w_k, w_v, w_o):
    from concourse.bass_utils import run_bass_kernel_spmd

    nc = get_nc()
    in_maps = make_in_maps(x, positions, w_q, w_k, w_v, w_o)
    res = run_bass_kernel_spmd(nc, in_maps, core_ids=list(range(N_CORES)))
    return assemble_output(res.results)


# revision 33
# speedup vs baseline: 1.1544x; 1.1223x over previous
"""Distributed GQA attention prefill for TRN2 (8 NeuronCores).

Problem: T=2048, D=4096, N=32 query heads, K=8 kv heads, H=128.
    q = x @ w_q; k = x @ w_k; v = x @ w_v   (fused in the reference)
    rope(q), rope(k); causal GQA attention; out = o @ w_o

Sharding (tensor-parallel over heads): core c owns query heads
4c..4c+3 and kv head c (GQA groups align: query heads 4c..4c+3 attend
kv head c). w_q/w_o sharded on N, w_k/w_v on K, x replicated. Each
core computes its partial o_proj output [T, D]; a bf16 ReduceScatter
(pipelined per 512-row t-panel) sums partials; the host concatenates
the per-core shards.

On-core dataflow (bf16 matmuls, fp32 PSUM accumulation):
    x --cast-DMA--> bf16 --DMA-xbar-transpose--> xT[d, t] (SBUF)
    qT/kT = w.T @ xT per head (h on partitions), RoPE applied in the
    transposed layout with host-precomputed cos/sin tables (the half
    swap is an SBUF->SBUF DMA).
    S^T[s, t] = kT_sblock.T @ qT_panel  (causal block skipping)
    P^T = exp(S^T / sqrt(H)) on ScalarE (scores are O(1); no max pass)
    PV: out[t, 0:129] = P^T_block.T @ [v | ones]  (row sums for free)
    normalize with DVE reciprocal + tensor_scalar, transpose o via
    identity matmul, o_proj accumulating over heads, per-panel RS.
"""

import numpy as np
import ml_dtypes

T, D, NH, KH, H = 2048, 4096, 32, 8, 128
THETA = 10000.0
G = NH // KH          # 4 query heads per core
N_CORES = 8
TP_SIZE = 512         # t-panel
NTP = T // TP_SIZE    # 4 t-panels
NTB = T // 128        # 16 t/s blocks
NDB = D // 128        # 32 d blocks
SCALE = 1.0 / float(np.sqrt(H))
VEXT_STRIDE = 132     # v_ext row stride (129 used, padded)
# Output ReduceScatter chunks: (global_start_row, nrows). Large chunks for
# stream throughput; small final chunks to shrink the serial tail.
RS_CHUNKS = [(0, 256), (256, 256), (512, 256), (768, 256), (1024, 256),
             (1280, 256), (1536, 256), (1792, 128), (1920, 128)]

_NC_CACHE = {}


def _build_nc():
    import concourse.mybir as mybir
    import concourse.tile as tile
    from concourse import bacc
    from concourse.masks import make_identity

    BF16 = mybir.dt.bfloat16
    F32 = mybir.dt.float32
    EXP = mybir.ActivationFunctionType.Exp

    nc = bacc.Bacc("TRN2", target_bir_lowering=False, debug=False,
                   num_devices=N_CORES)

    xt_ext = nc.dram_tensor("xt", [128, NTP, NDB, TP_SIZE], BF16,
                            kind="ExternalInput")
    wq_ext = nc.dram_tensor("wq", [128, NDB, G * H], BF16,
                            kind="ExternalInput")
    wk_ext = nc.dram_tensor("wk", [128, NDB, H], BF16, kind="ExternalInput")
    wv_ext = nc.dram_tensor("wv", [128, NDB, H], BF16, kind="ExternalInput")
    wo_ext = nc.dram_tensor("wo", [128, G, D], BF16, kind="ExternalInput")
    cos_ext = nc.dram_tensor("cos_t", [H, T], BF16, kind="ExternalInput")
    sin_ext = nc.dram_tensor("sin_t", [H, T], BF16, kind="ExternalInput")
    mask_ext = nc.dram_tensor("maskp", [128, 128], BF16, kind="ExternalInput")
    out_ext = nc.dram_tensor("out", [T // N_CORES, D], BF16,
                             kind="ExternalOutput")

    with tile.TileContext(nc) as tc:
        with (
            tc.tile_pool(name="consts", bufs=1) as consts,
            tc.tile_pool(name="persist", bufs=1) as persist,
            tc.tile_pool(name="dram", bufs=1, space="DRAM") as dram,
        ):
            cos_sb = consts.tile([H, T], BF16)
            sin_sb = consts.tile([H, T], BF16)
            mask_sb = consts.tile([128, 128], BF16)
            ident = consts.tile([128, 128], BF16)
            make_identity(nc, ident[:])

            qT = [persist.tile([128, T], BF16, tag=f"qT{g}", name=f"qT{g}")
                  for g in range(G)]
            kT = persist.tile([128, T], BF16)
            v_ext = persist.tile([128, NTB, VEXT_STRIDE], BF16)

            rs_in = [dram.tile([n, D], BF16, tag=f"rsi{ch}", name=f"rsi{ch}")
                     for ch, (s, n) in enumerate(RS_CHUNKS)]
            rs_out = [dram.tile([n // N_CORES, D], BF16, tag=f"rso{ch}",
                                name=f"rso{ch}")
                      for ch, (s, n) in enumerate(RS_CHUNKS)]

            # ============ phase A: x transpose + QKV + RoPE ============
            with (
                tc.tile_pool(name="xw", bufs=1) as xw,
                tc.tile_pool(name="xtpp", bufs=2) as xtpp,
                tc.tile_pool(name="rawp", bufs=1) as rawp,
                tc.tile_pool(name="ropep", bufs=1) as ropep,
                tc.tile_pool(name="qkvps", bufs=4, space="PSUM") as qkv_ps,
                tc.tile_pool(name="vtrps", bufs=2, space="PSUM") as vtr_ps,
            ):
                wq_sb = xw.tile([128, NDB, G * H], BF16)
                wk_sb = xw.tile([128, NDB, H], BF16)
                wv_sb = xw.tile([128, NDB, H], BF16)
                vraw = rawp.tile([128, T], BF16, tag="raw")

                nc.scalar.dma_start(out=wk_sb[:], in_=wk_ext[:])
                nc.scalar.dma_start(out=wv_sb[:], in_=wv_ext[:])
                for cch in range(4):
                    nc.scalar.dma_start(
                        out=wq_sb[:, 8 * cch:8 * (cch + 1), :],
                        in_=wq_ext[:, 8 * cch:8 * (cch + 1), :])
                nc.gpsimd.dma_start(out=cos_sb[:], in_=cos_ext[:])
                nc.gpsimd.dma_start(out=sin_sb[:], in_=sin_ext[:])
                nc.gpsimd.dma_start(out=mask_sb[:], in_=mask_ext[:])

                def rope(raw, dst):
                    """dst = raw*cos + halfswap(raw)*sin (full [128, T])."""
                    sw = ropep.tile([128, T], BF16, tag="ropesw")
                    t1 = ropep.tile([128, T], BF16, tag="ropet1")
                    nc.gpsimd.dma_start(out=sw[0:64, :], in_=raw[64:128, :])
                    nc.gpsimd.dma_start(out=sw[64:128, :], in_=raw[0:64, :])
                    nc.vector.tensor_tensor(
                        out=t1[:], in0=raw[:], in1=cos_sb[:],
                        op=mybir.AluOpType.mult)
                    nc.vector.tensor_tensor(
                        out=sw[:], in0=sw[:], in1=sin_sb[:],
                        op=mybir.AluOpType.mult)
                    nc.vector.tensor_tensor(
                        out=dst[:], in0=t1[:], in1=sw[:],
                        op=mybir.AluOpType.add)

                for tp in range(NTP):
                    xTp = xtpp.tile([128, NDB, TP_SIZE], BF16, tag="xTp")
                    if tp == 0:
                        for dc in range(4):
                            nc.sync.dma_start(
                                out=xTp[:, 8 * dc:8 * (dc + 1), :],
                                in_=xt_ext[:, 0, 8 * dc:8 * (dc + 1), :])
                    else:
                        nc.sync.dma_start(out=xTp[:], in_=xt_ext[:, tp])
                    tsl = slice(tp * TP_SIZE, (tp + 1) * TP_SIZE)
                    ps = qkv_ps.tile([128, TP_SIZE], F32, tag="qkv")
                    for db in range(NDB):
                        nc.tensor.matmul(
                            ps[:], wk_sb[:, db, :], xTp[:, db, :],
                            start=(db == 0), stop=(db == NDB - 1))
                    nc.scalar.copy(kT[:, tsl], ps[:])
                    ps = qkv_ps.tile([128, TP_SIZE], F32, tag="qkv")
                    for db in range(NDB):
                        nc.tensor.matmul(
                            ps[:], wv_sb[:, db, :], xTp[:, db, :],
                            start=(db == 0), stop=(db == NDB - 1))
                    nc.scalar.copy(vraw[:, tsl], ps[:])
                    for g in range(G):
                        ps = qkv_ps.tile([128, TP_SIZE], F32, tag="qkv")
                        for db in range(NDB):
                            nc.tensor.matmul(
                                ps[:],
                                wq_sb[:, db, g * H:(g + 1) * H],
                                xTp[:, db, :],
                                start=(db == 0), stop=(db == NDB - 1))
                        nc.scalar.copy(qT[g][:, tsl], ps[:])

                rope(kT, kT)
                for g in range(G):
                    rope(qT[g], qT[g])
                nc.vector.memset(v_ext[:, :, 128:129], 1.0)
                for sb in range(NTB):
                    pst = vtr_ps.tile([128, 128], BF16, tag="vtr")
                    nc.tensor.transpose(
                        pst[:], vraw[:, sb * 128:(sb + 1) * 128], ident[:])
                    nc.vector.tensor_copy(v_ext[:, sb, 0:128], pst[:])

            # ============ phase B: attention + o_proj + RS ============
            with (
                tc.tile_pool(name="wop", bufs=1) as wop,
                tc.tile_pool(name="ptp", bufs=20) as ptp,
                tc.tile_pool(name="otp", bufs=8) as otp,
                tc.tile_pool(name="osbp", bufs=4) as osbp,
                tc.tile_pool(name="scp", bufs=6) as scp,
                tc.tile_pool(name="sps", bufs=2, space="PSUM") as sps,
                tc.tile_pool(name="smallps", bufs=2, space="PSUM") as smallps,
                tc.tile_pool(name="oprojps", bufs=4, space="PSUM") as oprojps,
            ):
                wo_sb = wop.tile([128, G, D], BF16)
                nc.gpsimd.dma_start(out=wo_sb[:], in_=wo_ext[:])

                def oproj(tp, j, oT_t):
                    """o_proj for t-block j of panel tp + its ReduceScatter."""
                    tb = 4 * tp + j
                    ch = next(i for i, (s, n) in enumerate(RS_CHUNKS)
                              if s <= tb * 128 < s + n)
                    row = tb * 128 - RS_CHUNKS[ch][0]
                    for dh in range(2):
                        osb = osbp.tile([128, D // 2], BF16, tag="osb")
                        ods = [oprojps.tile([128, 512], F32, tag="od",
                                            name=f"od{tp}_{j}_{dh}_{ii}")
                               for ii in range(4)]
                        for g in range(G):
                            for i, dp in enumerate(range(dh * 4, dh * 4 + 4)):
                                nc.tensor.matmul(
                                    ods[i][:],
                                    oT_t[g][:, j * 128:(j + 1) * 128],
                                    wo_sb[:, g, dp * 512:(dp + 1) * 512],
                                    start=(g == 0), stop=(g == G - 1),
                                    skip_group_check=True)
                        for i, dp in enumerate(range(dh * 4, dh * 4 + 4)):
                            if tp == NTP - 1 and dp % 2 == 1:
                                nc.scalar.copy(
                                    osb[:, (dp - dh * 4) * 512:
                                        (dp - dh * 4 + 1) * 512], ods[i][:])
                            else:
                                nc.vector.tensor_copy(
                                    osb[:, (dp - dh * 4) * 512:
                                        (dp - dh * 4 + 1) * 512], ods[i][:])
                        nc.sync.dma_start(
                            out=rs_in[ch][row:row + 128,
                                          dh * 2048:(dh + 1) * 2048],
                            in_=osb[:])
                    if row + 128 == RS_CHUNKS[ch][1]:
                        nc.gpsimd.collective_compute(
                            "ReduceScatter",
                            mybir.AluOpType.add,
                            replica_groups=[list(range(N_CORES))],
                            ins=[rs_in[ch].opt()],
                            outs=[rs_out[ch].opt()],
                        )
                        s, n = RS_CHUNKS[ch]
                        nc.gpsimd.dma_start(
                            out=out_ext[s // N_CORES:
                                        s // N_CORES + n // N_CORES, :],
                            in_=rs_out[ch][:])

                for tp in range(NTP):
                    n_sb = 4 * tp + 4
                    oT_t = [otp.tile([128, TP_SIZE], BF16, tag="oT",
                                     name=f"oT{tp}_{gg}")
                            for gg in range(G)]
                    for g in range(G):
                        pts = []
                        for sb in range(n_sb):
                            ps_s = sps.tile([128, TP_SIZE], F32, tag="s")
                            nc.tensor.matmul(
                                ps_s[:], kT[:, sb * 128:(sb + 1) * 128],
                                qT[g][:, tp * TP_SIZE:(tp + 1) * TP_SIZE],
                                start=True, stop=True)
                            pt = ptp.tile([128, TP_SIZE], BF16, tag="pt")
                            nc.scalar.activation(pt[:], ps_s[:], EXP,
                                                 scale=SCALE)
                            jj = sb - 4 * tp
                            if 0 <= jj:
                                nc.vector.tensor_tensor(
                                    out=pt[:, jj * 128:(jj + 1) * 128],
                                    in0=pt[:, jj * 128:(jj + 1) * 128],
                                    in1=mask_sb[:],
                                    op=mybir.AluOpType.mult)
                            pts.append(pt)
                        for j in range(4):
                            tb = 4 * tp + j
                            ps_pv = smallps.tile([128, 129], F32, tag="sm")
                            for sb in range(tb + 1):
                                nc.tensor.matmul(
                                    ps_pv[:],
                                    pts[sb][:, j * 128:(j + 1) * 128],
                                    v_ext[:, sb, 0:129],
                                    start=(sb == 0), stop=(sb == tb),
                                    skip_group_check=True)
                            rc = scp.tile([128, 1], F32, tag="rc")
                            nc.vector.reciprocal(rc[:], ps_pv[:, 128:129])
                            ob = scp.tile([128, 128], BF16, tag="ob")
                            nc.vector.tensor_scalar_mul(
                                ob[:], ps_pv[:, 0:128], rc[:])
                            ps_tr = smallps.tile([128, 128], BF16, tag="sm")
                            nc.tensor.transpose(ps_tr[:], ob[:], ident[:])
                            nc.vector.tensor_copy(
                                oT_t[g][:, j * 128:(j + 1) * 128], ps_tr[:])
                            if g == G - 1:
                                oproj(tp, j, oT_t)

    nc.compile()
    return nc


def get_nc():
    if "nc" not in _NC_CACHE:
        _NC_CACHE["nc"] = _build_nc()
    return _NC_CACHE["nc"]


def make_in_maps(x, positions, w_q, w_k, w_v, w_o):
    """Host-side sharding + RoPE table / mask precompute."""
    x = np.ascontiguousarray(np.asarray(x, np.float32))
    positions = np.asarray(positions)

    half = H // 2
    inv_freq = 1.0 / (THETA ** (np.arange(half, dtype=np.float32) / half))
    ang = positions.astype(np.float32)[:, None] * inv_freq[None, :]  # [T, 64]
    cos = np.cos(ang)   # [T, 64]
    sin = np.sin(ang)
    cos_t = np.empty((H, T), np.float32)
    sin_t = np.empty((H, T), np.float32)
    cos_t[0:half] = cos.T
    cos_t[half:] = cos.T
    sin_t[0:half] = -sin.T
    sin_t[half:] = sin.T
    cos_t = cos_t.astype(ml_dtypes.bfloat16)
    sin_t = sin_t.astype(ml_dtypes.bfloat16)

    # mask[s, t] = 1 if s <= t (lower-left of P^T allowed region)
    idx = np.arange(128)
    maskp = (idx[:, None] <= idx[None, :]).astype(ml_dtypes.bfloat16)

    xt = x.astype(ml_dtypes.bfloat16).T  # [D, T]
    xt4 = np.ascontiguousarray(
        xt.reshape(NDB, 128, NTP, TP_SIZE).transpose(1, 2, 0, 3))
    w_q = np.asarray(w_q, np.float32).reshape(D, NH, H).astype(
        ml_dtypes.bfloat16)
    w_k = np.asarray(w_k, np.float32).reshape(D, KH, H).astype(
        ml_dtypes.bfloat16)
    w_v = np.asarray(w_v, np.float32).reshape(D, KH, H).astype(
        ml_dtypes.bfloat16)
    w_o = np.asarray(w_o, np.float32).reshape(NH, H, D).astype(
        ml_dtypes.bfloat16)

    def blk(w):
        """[D, n] -> [128, NDB, n] with row d = a*128 + p."""
        return np.ascontiguousarray(
            w.reshape(NDB, 128, -1).transpose(1, 0, 2))

    in_maps = []
    for c in range(N_CORES):
        in_maps.append({
            "xt": xt4,
            "wq": blk(w_q[:, G * c:G * (c + 1), :].reshape(D, G * H)),
            "wk": blk(w_k[:, c, :]),
            "wv": blk(w_v[:, c, :]),
            "wo": np.ascontiguousarray(
                w_o[G * c:G * (c + 1)].reshape(G, 128, D)
                .transpose(1, 0, 2)),
            "cos_t": cos_t,
            "sin_t": sin_t,
            "maskp": maskp,
        })
    return in_maps


def assemble_output(results):
    """results: list of 8 per-core dicts with 'out' [T//8, D] bf16."""
    out = np.empty((T, D), np.float32)
    for c in range(N_CORES):
        o = np.asarray(results[c]["out"], np.float32)
        for s, n in RS_CHUNKS:
            k = n // N_CORES
            out[s + c * k:s + (c + 1) * k] = o[s // N_CORES:s // N_CORES + k]
    return out


def kernel(x, positions, w_q, w_k, w_v, w_o):
    from concourse.bass_utils import run_bass_kernel_spmd

    nc = get_nc()
    in_maps = make_in_maps(x, positions, w_q, w_k, w_v, w_o)
    res = run_bass_kernel_spmd(nc, in_maps, core_ids=list(range(N_CORES)))
    return assemble_output(res.results)
